# revision 37
# baseline (speedup 1.0000x reference)
"""GroupedPNMLP forward on 8 Trainium2 NeuronCores (pure data parallel).

Per-node 2-layer MLP (32->32->1), 24 nodes in 6 groups of 4, with a
group-validity mask and node permutation.  Full inputs in, full output out;
samples are sharded N/8 per core, tiny weights replicated.

Per-core: 3-stage software pipeline over chunks (small 1024-sample chunks
at both ends to shorten fill/drain, 2048 in steady state):
  stage A (it):   DMA h halves + packed valid (sync HWDGE ring)
  stage B (it-1): DVE-only ReLU + fp32->bf16 (2x_2P mode) and 32x32
                  block-transpose to channel-major; group mask = one DVE
                  max-reduce (all engines join-free: ACT never feeds DVE)
  stage C (it-2): mm1 = 16 concurrent 32x32 bf16 sub-tile matmuls per
                  group-wave (tile_position, weights replicated across the
                  4 sample strips); ACT fused ReLU+b1 PSUM->SBUF (bf16);
                  mm2 block-W2 accumulated over 6 waves into one PSUM
                  bank; b2 via a rank-1 matmul; DVE transpose PSUM->SBUF;
                  mask multiply via broadcast AP; packed store

All node-indexed lanes run in GROUPING order (wave w = group w); the host
permutes valid columns on pack and un-permutes output columns on unpack.
"""

import numpy as np
import ml_dtypes

import concourse.bass as bass
from concourse import bacc
import concourse.tile as tile
from concourse import mybir
from concourse.bass_utils import run_bass_kernel_spmd

F32 = mybir.dt.float32
BF16 = mybir.dt.bfloat16
I32 = mybir.dt.int32

GROUPING = np.array(
    [[0, 3, 6, 9], [1, 4, 7, 10], [2, 5, 8, 11],
     [12, 13, 14, 15], [16, 18, 20, 22], [17, 19, 21, 23]], dtype=np.int32)
GFLAT = GROUPING.reshape(-1)

N_CORES = 8
S_TOT = 131072
S = S_TOT // N_CORES      # 16384 samples per core
NODES = 24
CH = 32                   # in channels = hidden dim
NW = 6                    # waves = groups
TOTSUB = S // 128         # 128 sub-tiles of 128 samples per core
CSIZES = [1024, 1024] + [2048] * 7
assert sum(CSIZES) == S


def _build_program():
    nc = bacc.Bacc(None, target_bir_lowering=False)

    h = nc.dram_tensor("h", [S, NODES * CH], BF16, kind="ExternalInput")
    vpk = nc.dram_tensor("vpk", [128, TOTSUB, NODES], I32,
                         kind="ExternalInput")
    w1rep = nc.dram_tensor("w1rep", [128, NW, 4, CH], BF16,
                           kind="ExternalInput")
    w2blk = nc.dram_tensor("w2blk", [128, NW, 32], BF16, kind="ExternalInput")
    b1col = nc.dram_tensor("b1col", [128, NW], F32, kind="ExternalInput")
    b2row = nc.dram_tensor("b2row", [1, 128], BF16, kind="ExternalInput")
    out = nc.dram_tensor("out", [128, TOTSUB, 32], F32, kind="ExternalOutput")

    with tile.TileContext(nc) as tc:
        with (
            tc.tile_pool(name="singles", bufs=1) as singles,
            tc.tile_pool(name="xp", bufs=6) as xp,
            tc.tile_pool(name="xtp", bufs=3) as xtp,
            tc.tile_pool(name="hidp", bufs=2) as hidp,
            tc.tile_pool(name="vp", bufs=4) as vp,
            tc.tile_pool(name="op", bufs=2) as op,
            tc.tile_pool(name="pha_pool", bufs=2, space="PSUM") as pha_pool,
            tc.tile_pool(name="phb_pool", bufs=1, space="PSUM") as phb_pool,
            tc.tile_pool(name="p2_pool", bufs=2, space="PSUM") as p2_pool,
        ):
            w1sb = singles.tile([128, NW, 4, CH], BF16)
            nc.gpsimd.dma_start(out=w1sb, in_=w1rep[:, :, :, :])
            w2sb = singles.tile([128, NW, 32], BF16)
            nc.gpsimd.dma_start(out=w2sb, in_=w2blk[:, :, :])
            b1sb = singles.tile([128, NW], F32)
            nc.gpsimd.dma_start(out=b1sb, in_=b1col[:, :])
            b2sb = singles.tile([1, 128], BF16)
            nc.gpsimd.dma_start(out=b2sb, in_=b2row[:, :])
            ones = singles.tile([1, 512], BF16)
            nc.vector.memset(ones, 1.0)

            def emit_loads(cc):
                cs = CSIZES[cc]
                c0 = sum(CSIZES[:cc])
                hsub = cs // 256          # sub-tiles per half-chunk
                xhs = []
                for hh in range(2):
                    xh = xp.tile([128, hsub, NODES, CH], BF16, tag="xh")
                    lo = c0 + hh * (cs // 2)
                    nc.sync.dma_start(
                        out=xh.rearrange("p s n c -> p s (n c)"),
                        in_=h[lo:lo + cs // 2, :].rearrange(
                            "(s p) f -> p s f", p=128),
                    )
                    xhs.append(xh)
                vi = vp.tile([128, cs // 128, NODES], I32, tag="vi")
                nc.sync.dma_start(
                    out=vi, in_=vpk[:, c0 // 128:(c0 + cs) // 128])
                return {"xhs": xhs, "vi": vi, "cs": cs, "c0": c0}

            def emit_input(st):
                cs = st["cs"]
                nsub = cs // 128
                hsub = cs // 256
                # in-place bf16 relu (DVE 4x mode), then 32x32 transpose
                xt = xtp.tile([128, nsub, NODES, CH], BF16, tag="xt")
                for hh in range(2):
                    xh = st["xhs"][hh]
                    xhf = xh.rearrange("p s n c -> p (s n c)")
                    nc.vector.tensor_scalar(
                        xhf, xhf, 0.0, None, op0=mybir.AluOpType.max)
                    nc.vector.transpose(
                        xt[:, hh * hsub:(hh + 1) * hsub], xh)
                st["xt"] = xt

                # group mask (valid cols pre-permuted to group order on host)
                gv = vp.tile([128, nsub, NW], F32, tag="gv")
                nc.vector.tensor_reduce(
                    gv,
                    st["vi"].rearrange("p s (g k) -> p s g k", k=4),
                    axis=mybir.AxisListType.X, op=mybir.AluOpType.max)
                st["gv"] = gv

            def emit_mm(st):
                cs = st["cs"]
                nsub = cs // 128
                cq = cs // 4
                xt = st["xt"]
                p2 = p2_pool.tile([128, nsub, 32], F32, tag="p2")
                p2f = p2.rearrange("p a b -> p (a b)")
                for w in range(NW):
                    # fixed 2-bank tiles: strip i always gets its own PSUM
                    # bank (fo step 512) even when cq < 512
                    pha = pha_pool.tile([128, 1024], F32, tag="pha")
                    phb = phb_pool.tile([128, 1024], F32, tag="phb")
                    for i in range(4):
                        ph_t = pha if i < 2 else phb
                        fo = (i % 2) * 512
                        for jj in range(4):
                            n = int(GROUPING[w][jj])
                            nc.tensor.matmul(
                                ph_t[32 * jj:32 * jj + 32, fo:fo + cq],
                                lhsT=w1sb[32 * i:32 * i + 32, w, jj, :],
                                rhs=xt[32 * i:32 * i + 32, :, n, :],
                                start=True, stop=True,
                                tile_position=(32 * i, 32 * jj))
                    hid = hidp.tile([128, cs], BF16, tag="hid")
                    nc.scalar.activation(
                        hid[:, 0:2 * cq].rearrange(
                            "p (two q) -> p two q", two=2),
                        pha.rearrange(
                            "p (two f) -> p two f", two=2)[:, :, 0:cq],
                        mybir.ActivationFunctionType.Relu,
                        bias=b1sb[:, w:w + 1])
                    nc.scalar.activation(
                        hid[:, 2 * cq:4 * cq].rearrange(
                            "p (two q) -> p two q", two=2),
                        phb.rearrange(
                            "p (two f) -> p two f", two=2)[:, :, 0:cq],
                        mybir.ActivationFunctionType.Relu,
                        bias=b1sb[:, w:w + 1])
                    for i in range(4):
                        nc.tensor.matmul(
                            p2f[32 * i:32 * i + 32, :],
                            lhsT=w2sb[:, w, :],
                            rhs=hid[:, i * cq:(i + 1) * cq],
                            start=(w == 0), stop=False,
                            skip_group_check=True,
                            tile_position=(0, 32 * i))
                # += b2 (rank-1 matmul: b2 per out-partition x ones row)
                nc.tensor.matmul(
                    p2f,
                    lhsT=b2sb[:, :],
                    rhs=ones[:, 0:cq],
                    start=False, stop=True,
                    skip_group_check=True,
                    tile_position=(0, 0))
                st["p2"] = p2

            def emit_tail(st):
                cs = st["cs"]
                nsub = cs // 128
                # transpose back (PSUM src), mask via broadcast AP, store
                outT = op.tile([128, nsub, 32], F32, tag="outT")
                nc.vector.transpose(outT, st["p2"])
                o4 = outT[:, :, 0:NODES].rearrange("p s (g k) -> p s g k", k=4)
                nc.vector.tensor_tensor(
                    o4, o4, st["gv"].broadcast_to([128, nsub, NW, 4]),
                    op=mybir.AluOpType.mult)
                sub0 = st["c0"] // 128
                nc.sync.dma_start(out=out[:, sub0:sub0 + nsub], in_=outT)

            # 4-stage pipeline: loads(k) | input(k-1) | mm(k-2) | tail(k-3)
            nch = len(CSIZES)
            st = [None] * nch
            for it in range(nch + 3):
                if it < nch:
                    st[it] = emit_loads(it)
                if 1 <= it <= nch:
                    emit_input(st[it - 1])
                if 2 <= it <= nch + 1:
                    emit_mm(st[it - 2])
                if 3 <= it <= nch + 2:
                    emit_tail(st[it - 3])
                    st[it - 3] = None


    nc.compile()
    return nc


_PROGRAM = None


def _get_program():
    global _PROGRAM
    if _PROGRAM is None:
        _PROGRAM = _build_program()
    return _PROGRAM


def _prep_weights(W1, b1, W2, b2):
    W1n = np.asarray(W1, np.float32)   # [6, 4, 32, 32] group-major already
    W2n = np.asarray(W2, np.float32)   # [6, 4, 32, 1]
    b1n = np.asarray(b1, np.float32)   # [6, 4, 32]
    b2n = np.asarray(b2, np.float32)   # [6, 4, 1]

    w1rep = np.zeros((128, NW, 4, CH), np.float32)
    w2blk = np.zeros((128, NW, 32), np.float32)
    b1col = np.zeros((128, NW), np.float32)
    b2row = np.zeros((1, 128), np.float32)
    for w in range(NW):
        for jj in range(4):
            q = 4 * w + jj
            for r in range(4):
                w1rep[32 * r:32 * r + 32, w, jj, :] = W1n[w, jj]
            w2blk[32 * jj:32 * jj + 32, w, q] = W2n[w, jj, :, 0]
            b1col[32 * jj:32 * jj + 32, w] = b1n[w, jj]
    for i in range(4):
        b2row[0, 32 * i:32 * i + 24] = b2n.reshape(-1)
    return (w1rep.astype(ml_dtypes.bfloat16),
            w2blk.astype(ml_dtypes.bfloat16),
            b1col,
            b2row.astype(ml_dtypes.bfloat16))


def _make_in_maps(inputs):
    w1rep, w2blk, b1col, b2row = _prep_weights(
        inputs["W1"], inputs["b1"], inputs["W2"], inputs["b2"])
    h2 = np.ascontiguousarray(np.asarray(
        inputs["h"], dtype=np.float32).astype(ml_dtypes.bfloat16)).reshape(
        S_TOT, NODES * CH)
    v2 = np.asarray(inputs["valid"], dtype=np.int32).reshape(S_TOT, NODES)
    v2g = np.ascontiguousarray(v2[:, GFLAT])  # group-major columns

    in_maps = []
    for c in range(N_CORES):
        sl = slice(c * S, (c + 1) * S)
        # pack valid partition-major: vpk[p, sub, q] = v[128*sub + p, q]
        vpk = np.ascontiguousarray(
            v2g[sl].reshape(TOTSUB, 128, NODES).transpose(1, 0, 2))
        in_maps.append({
            "h": h2[sl],
            "vpk": vpk,
            "w1rep": w1rep,
            "w2blk": w2blk,
            "b1col": b1col,
            "b2row": b2row,
        })
    return in_maps


def kernel(h, valid, W1, b1, W2, b2):
    nc = _get_program()
    in_maps = _make_in_maps(dict(h=h, valid=valid, W1=W1, b1=b1, W2=W2, b2=b2))
    res = run_bass_kernel_spmd(nc, in_maps, core_ids=list(range(N_CORES)))
    outs = []
    for c in range(N_CORES):
        arr = res.results[c]["out"]  # [128, TOTSUB, 32], group-major cols
        flat = np.ascontiguousarray(
            arr.transpose(1, 0, 2)).reshape(S, 32)[:, :NODES]
        un = np.empty((S, NODES), np.float32)
        un[:, GFLAT] = flat          # q -> node id
        outs.append(un)
    full = np.concatenate(outs, axis=0).astype(np.float32)
    return full.reshape(S_TOT, NODES, 1)
